# revision 9
# baseline (speedup 1.0000x reference)
"""Trainium2 Bass kernel for nn_CompressModel: y = FHT_1024(x * golay) / (alpha + eps).

Factorization: H_1024 = H_8 (outer, feat bits 7-9) (x) H_128 (inner, feat bits 0-6).

Per-core dataflow (pure data-parallel over rows; 4096 rows/core, blocks of 128 rows):
  1. DMA x block [128 rows, 1024] -> SBUF [128p, 1024f]  (straight, 4KB descriptors)
  2. PE transposes each [128r, 128b] a-block -> PSUM zt [b, (a r)]  (2 banks of 4)
  3. ScalarE drains PSUM->SBUF bf16 with 1/(alpha+eps) per-partition scale
  4. PE matmul w_a[r,b'] = zb_a[b,r].T @ Ha[b,b'] in bf16 (1 cyc/row), where
     Ha = golay(a*128+b) * H128  -- golay multiply folded into 8 stationary mats
  5. H_8 = 3 butterfly levels over a-blocks in row space:
     L1 fused with PSUM drain on DVE (fp32->bf16), L2 on DVE (bf16 4x), L3 on
     GpSimd (bf16->fp32 out)
  6. DMA out.

All engines sit well below the ~93us/core DMA roofline (33.5 MB @ 360 GB/s), so
the 16 DMA engines stream continuously.
"""

import numpy as np
from contextlib import ExitStack

import ml_dtypes

import concourse.bass as bass
import concourse.tile as tile
from concourse import bacc, mybir
from concourse.bass_utils import run_bass_kernel_spmd

f32 = mybir.dt.float32
bf16 = mybir.dt.bfloat16

N_CORES = 8
DIM = 1024
EPS = 1e-5
ROWS_TOTAL = 4 * 8192          # 32768
ROWS_PER_CORE = ROWS_TOTAL // N_CORES   # 4096
BLK = 128                      # rows per block
N_BLK = ROWS_PER_CORE // BLK   # 32

LAST_RESULT = None  # test harness reads exec_time_ns from here


def _hadamard(n: int) -> np.ndarray:
    h = np.array([[1.0]], dtype=np.float32)
    while h.shape[0] < n:
        h = np.block([[h, h], [h, -h]])
    return np.ascontiguousarray(h.astype(np.float32))


def _build_nc():
    nc = bacc.Bacc("TRN2", target_bir_lowering=False, debug=False)
    x_d = nc.dram_tensor("x", [ROWS_PER_CORE, DIM], f32, kind="ExternalInput")
    s_d = nc.dram_tensor("svec", [128, 1], f32, kind="ExternalInput")
    h_d = nc.dram_tensor("hmats", [128, 8 * 128], bf16, kind="ExternalInput")
    i_d = nc.dram_tensor("ident", [128, 128], f32, kind="ExternalInput")
    y_d = nc.dram_tensor("y", [ROWS_PER_CORE, DIM], f32, kind="ExternalOutput")

    with TileKernel(nc) as tk:
        tk.emit(x_d, s_d, h_d, i_d, y_d)

    nc.compile()
    return nc


class TileKernel:
    def __init__(self, nc):
        self.nc = nc
        self.ctx = ExitStack()

    def __enter__(self):
        self.tc = self.ctx.enter_context(tile.TileContext(self.nc))
        return self

    def __exit__(self, *exc):
        return self.ctx.__exit__(*exc)

    def emit(self, x_d, s_d, h_d, i_d, y_d):
        nc, tc, ctx = self.nc, self.tc, self.ctx

        const_pool = ctx.enter_context(tc.tile_pool(name="const", bufs=1))
        x_pool = ctx.enter_context(tc.tile_pool(name="x", bufs=6))
        zb_pool = ctx.enter_context(tc.tile_pool(name="zb", bufs=4))
        v_pool = ctx.enter_context(tc.tile_pool(name="v", bufs=3))
        u_pool = ctx.enter_context(tc.tile_pool(name="u", bufs=3))
        w1s_pool = ctx.enter_context(tc.tile_pool(name="w1s", bufs=3))
        y_pool = ctx.enter_context(tc.tile_pool(name="y", bufs=6))
        zt_pool = ctx.enter_context(tc.tile_pool(name="zt", bufs=2, space="PSUM"))
        w_pool = ctx.enter_context(tc.tile_pool(name="w", bufs=2, space="PSUM"))

        ident = const_pool.tile([128, 128], f32)
        nc.sync.dma_start(ident[:], i_d.ap()[:, :])
        hm = const_pool.tile([128, 8 * 128], bf16)
        nc.sync.dma_start(hm[:], h_d.ap()[:, :])
        svec = const_pool.tile([128, 1], f32)
        nc.sync.dma_start(svec[:], s_d.ap()[:, :])

        for blk in range(N_BLK):
            r0 = blk * BLK
            # ---- load block: [128 rows, 1024] straight ----
            x_st = x_pool.tile([128, DIM], f32)
            nc.sync.dma_start(x_st[:], x_d.ap()[r0:r0 + BLK, :])

            # ---- transpose-in: 8 a-blocks -> one 2-bank PSUM tile [b, (a r)] ----
            zt = zt_pool.tile([128, DIM], f32)
            for a in range(8):
                nc.tensor.transpose(
                    zt[:, a * 128:(a + 1) * 128],
                    x_st[:, a * 128:(a + 1) * 128],
                    ident[:],
                )
            # single drain PSUM -> SBUF bf16 with 1/(alpha+eps) scale
            zb = zb_pool.tile([128, DIM], bf16)
            nc.scalar.mul(zb[:], zt[:], svec[:, 0:1])

            # ---- inner H_128 (golay folded into Ha) ----
            # even input blocks a -> w0 slot a//2, odd -> w1 slot a//2, so the
            # first butterfly level (over a-bit0) pairs across the two PSUM
            # tiles (DVE can read at most one PSUM operand).
            w0 = w_pool.tile([128, 512], f32)
            w1 = w_pool.tile([128, 512], f32)
            for a in range(8):
                w = w1 if a % 2 else w0
                j = a // 2
                nc.tensor.matmul(
                    w[:, j * 128:(j + 1) * 128],
                    lhsT=zb[:, a * 128:(a + 1) * 128],
                    rhs=hm[:, a * 128:(a + 1) * 128],
                    start=True, stop=True,
                )

            # ---- outer FHT_8 butterflies (row space) ----
            # drain w1 -> SBUF bf16 on ACT (one-PSUM-operand rule)
            w1s = w1s_pool.tile([128, 512], bf16)
            nc.scalar.copy(w1s[:], w1[:])

            # level over a-bit0: v[H=sign][q] = w0[q] +/- w1s[q]; dense out
            v = v_pool.tile([128, DIM], bf16)
            nc.vector.tensor_add(v[:, 0:512], w0[:], w1s[:])
            nc.vector.tensor_sub(v[:, 512:1024], w0[:], w1s[:])

            # level over a-bit1: q=(A2,h) pairs along h; u[H][A2][B1=sign]
            u = u_pool.tile([128, DIM], bf16)
            v5 = v[:].rearrange("p (H A h t) -> p H A h t", H=2, A=2, h=2)
            u5 = u[:].rearrange("p (H A h t) -> p H A h t", H=2, A=2, h=2)
            nc.vector.tensor_add(u5[:, :, :, 0:1, :], v5[:, :, :, 0:1, :], v5[:, :, :, 1:2, :])
            nc.vector.tensor_sub(u5[:, :, :, 1:2, :], v5[:, :, :, 0:1, :], v5[:, :, :, 1:2, :])

            # level over a-bit2: pairs along A2 (stride 256); y block index
            # a' = 4*s + 2*B1 + H, so each sign's output is a dense 512 half.
            y_st = y_pool.tile([128, DIM], f32)
            u6 = u[:].rearrange("p (H A B t) -> p B H A t", H=2, A=2, B=2)
            ya = y_st[:, 0:512].rearrange("p (B H s t) -> p B H s t", B=2, H=2, s=1)
            ys = y_st[:, 512:1024].rearrange("p (B H s t) -> p B H s t", B=2, H=2, s=1)
            nc.gpsimd.tensor_add(ya[:], u6[:, :, :, 0:1, :], u6[:, :, :, 1:2, :])
            nc.gpsimd.tensor_sub(ys[:], u6[:, :, :, 0:1, :], u6[:, :, :, 1:2, :])

            # ---- store ----
            nc.sync.dma_start(y_d.ap()[r0:r0 + BLK, :], y_st[:])


_NC = None


def _get_nc():
    global _NC
    if _NC is None:
        _NC = _build_nc()
    return _NC


def kernel(x, golay, alpha):
    global LAST_RESULT
    x_np = np.ascontiguousarray(np.asarray(x, dtype=np.float32).reshape(ROWS_TOTAL, DIM))
    golay_np = np.asarray(golay, dtype=np.float32).reshape(DIM)
    alpha_np = np.float32(np.asarray(alpha, dtype=np.float32))

    s = np.float32(1.0) / (alpha_np + np.float32(EPS))
    svec = np.full((128, 1), s, dtype=np.float32)
    h128 = _hadamard(128)
    # hm[b, a*128 + b'] = golay[a*128 + b] * H128[b, b']
    gmat = golay_np.reshape(8, 128)          # [a, b]
    # hm[b, a, b'] = gmat[a, b] * h128[b, b']
    hm = np.empty((128, 8, 128), dtype=np.float32)
    for a in range(8):
        hm[:, a, :] = gmat[a][:, None] * h128
    hm_bf16 = np.ascontiguousarray(hm.reshape(128, 8 * 128).astype(ml_dtypes.bfloat16))
    ident = np.ascontiguousarray(np.eye(128, dtype=np.float32))

    nc = _get_nc()
    in_maps = [
        {
            "x": x_np[c * ROWS_PER_CORE:(c + 1) * ROWS_PER_CORE],
            "svec": svec,
            "hmats": hm_bf16,
            "ident": ident,
        }
        for c in range(N_CORES)
    ]
    res = run_bass_kernel_spmd(nc, in_maps, core_ids=list(range(N_CORES)))
    LAST_RESULT = res
    y = np.concatenate([r["y"] for r in res.results], axis=0)
    return y.reshape(4, 8192, DIM)


# revision 12
# speedup vs baseline: 1.0134x; 1.0134x over previous
"""Trainium2 Bass kernel for nn_CompressModel: y = FHT_1024(x * golay) / (alpha + eps).

Factorization: H_1024 = H_8 (outer, feat bits 7-9) (x) H_128 (inner, feat bits 0-6).

Per-core dataflow (pure data-parallel over rows; 4096 rows/core, blocks of 128 rows):
  1. DMA x block [128 rows, 1024] -> SBUF [128p, 1024f]  (straight, 4KB descriptors)
  2. PE transposes each [128r, 128b] a-block -> PSUM zt [b, (a r)]  (2 banks of 4)
  3. ScalarE drains PSUM->SBUF bf16 with 1/(alpha+eps) per-partition scale
  4. PE matmul w_a[r,b'] = zb_a[b,r].T @ Ha[b,b'] in bf16 (1 cyc/row), where
     Ha = golay(a*128+b) * H128  -- golay multiply folded into 8 stationary mats
  5. H_8 = 3 butterfly levels over a-blocks in row space:
     L1 fused with PSUM drain on DVE (fp32->bf16), L2 on DVE (bf16 4x), L3 on
     GpSimd (bf16->fp32 out)
  6. DMA out.

All engines sit well below the ~93us/core DMA roofline (33.5 MB @ 360 GB/s), so
the 16 DMA engines stream continuously.
"""

import numpy as np
from contextlib import ExitStack

import ml_dtypes

import concourse.bass as bass
import concourse.tile as tile
from concourse import bacc, mybir
from concourse.bass_utils import run_bass_kernel_spmd

f32 = mybir.dt.float32
bf16 = mybir.dt.bfloat16

N_CORES = 8
DIM = 1024
EPS = 1e-5
ROWS_TOTAL = 4 * 8192          # 32768
ROWS_PER_CORE = ROWS_TOTAL // N_CORES   # 4096
BLK = 128                      # rows per block
N_BLK = ROWS_PER_CORE // BLK   # 32

LAST_RESULT = None  # test harness reads exec_time_ns from here


def _hadamard(n: int) -> np.ndarray:
    h = np.array([[1.0]], dtype=np.float32)
    while h.shape[0] < n:
        h = np.block([[h, h], [h, -h]])
    return np.ascontiguousarray(h.astype(np.float32))


def _build_nc():
    nc = bacc.Bacc("TRN2", target_bir_lowering=False, debug=False)
    x_d = nc.dram_tensor("x", [ROWS_PER_CORE, DIM], f32, kind="ExternalInput")
    s_d = nc.dram_tensor("svec", [128, 1], f32, kind="ExternalInput")
    h_d = nc.dram_tensor("hmats", [128, 8 * 128], bf16, kind="ExternalInput")
    i_d = nc.dram_tensor("ident", [128, 128], f32, kind="ExternalInput")
    y_d = nc.dram_tensor("y", [ROWS_PER_CORE, DIM], f32, kind="ExternalOutput")

    with TileKernel(nc) as tk:
        tk.emit(x_d, s_d, h_d, i_d, y_d)

    nc.compile()
    return nc


class TileKernel:
    def __init__(self, nc):
        self.nc = nc
        self.ctx = ExitStack()

    def __enter__(self):
        self.tc = self.ctx.enter_context(tile.TileContext(self.nc))
        return self

    def __exit__(self, *exc):
        return self.ctx.__exit__(*exc)

    def emit(self, x_d, s_d, h_d, i_d, y_d):
        nc, tc, ctx = self.nc, self.tc, self.ctx

        const_pool = ctx.enter_context(tc.tile_pool(name="const", bufs=1))
        x_pool = ctx.enter_context(tc.tile_pool(name="x", bufs=8))
        zb_pool = ctx.enter_context(tc.tile_pool(name="zb", bufs=4))
        v_pool = ctx.enter_context(tc.tile_pool(name="v", bufs=4))
        u_pool = ctx.enter_context(tc.tile_pool(name="u", bufs=4))
        w1s_pool = ctx.enter_context(tc.tile_pool(name="w1s", bufs=4))
        y_pool = ctx.enter_context(tc.tile_pool(name="y", bufs=8))
        zt_pool = ctx.enter_context(tc.tile_pool(name="zt", bufs=2, space="PSUM"))
        w_pool = ctx.enter_context(tc.tile_pool(name="w", bufs=2, space="PSUM"))

        ident = const_pool.tile([128, 128], f32)
        nc.sync.dma_start(ident[:], i_d.ap()[:, :])
        hm = const_pool.tile([128, 8 * 128], bf16)
        nc.sync.dma_start(hm[:], h_d.ap()[:, :])
        svec = const_pool.tile([128, 1], f32)
        nc.sync.dma_start(svec[:], s_d.ap()[:, :])

        # out-DMAs are issued LAG blocks late so their sem wait is already
        # satisfied when they reach the head of the in-order SP queue --
        # otherwise each out-DMA stalls the queue and blocks later in-DMAs.
        LAG = 2
        pending = []  # (y_tile, r0) awaiting out-DMA issue

        for blk in range(N_BLK):
            r0 = blk * BLK
            # ---- load block: [128 rows, 1024] straight ----
            x_st = x_pool.tile([128, DIM], f32)
            nc.sync.dma_start(x_st[:], x_d.ap()[r0:r0 + BLK, :])

            # ---- transpose-in: 8 a-blocks -> one 2-bank PSUM tile [b, (a r)] ----
            zt = zt_pool.tile([128, DIM], f32)
            for a in range(8):
                nc.tensor.transpose(
                    zt[:, a * 128:(a + 1) * 128],
                    x_st[:, a * 128:(a + 1) * 128],
                    ident[:],
                )
            # single drain PSUM -> SBUF bf16 with 1/(alpha+eps) scale
            zb = zb_pool.tile([128, DIM], bf16)
            nc.scalar.mul(zb[:], zt[:], svec[:, 0:1])

            # ---- inner H_128 (golay folded into Ha) ----
            # even input blocks a -> w0 slot a//2, odd -> w1 slot a//2, so the
            # first butterfly level (over a-bit0) pairs across the two PSUM
            # tiles (DVE can read at most one PSUM operand).
            w0 = w_pool.tile([128, 512], f32)
            w1 = w_pool.tile([128, 512], f32)
            for a in range(8):
                w = w1 if a % 2 else w0
                j = a // 2
                nc.tensor.matmul(
                    w[:, j * 128:(j + 1) * 128],
                    lhsT=zb[:, a * 128:(a + 1) * 128],
                    rhs=hm[:, a * 128:(a + 1) * 128],
                    start=True, stop=True,
                )

            # ---- outer FHT_8 butterflies (row space) ----
            # drain w1 -> SBUF bf16 on ACT (one-PSUM-operand rule)
            w1s = w1s_pool.tile([128, 512], bf16)
            nc.scalar.copy(w1s[:], w1[:])

            # level over a-bit0: v[H=sign][q] = w0[q] +/- w1s[q]; dense out
            v = v_pool.tile([128, DIM], bf16)
            nc.vector.tensor_add(v[:, 0:512], w0[:], w1s[:])
            nc.vector.tensor_sub(v[:, 512:1024], w0[:], w1s[:])

            # level over a-bit1: q=(A2,h) pairs along h; u[H][A2][B1=sign]
            u = u_pool.tile([128, DIM], bf16)
            v5 = v[:].rearrange("p (H A h t) -> p H A h t", H=2, A=2, h=2)
            u5 = u[:].rearrange("p (H A h t) -> p H A h t", H=2, A=2, h=2)
            nc.vector.tensor_add(u5[:, :, :, 0:1, :], v5[:, :, :, 0:1, :], v5[:, :, :, 1:2, :])
            nc.vector.tensor_sub(u5[:, :, :, 1:2, :], v5[:, :, :, 0:1, :], v5[:, :, :, 1:2, :])

            # level over a-bit2: pairs along A2 (stride 256); y block index
            # a' = 4*s + 2*B1 + H, so each sign's output is a dense 512 half.
            y_st = y_pool.tile([128, DIM], f32)
            u6 = u[:].rearrange("p (H A B t) -> p B H A t", H=2, A=2, B=2)
            ya = y_st[:, 0:512].rearrange("p (B H s t) -> p B H s t", B=2, H=2, s=1)
            ys = y_st[:, 512:1024].rearrange("p (B H s t) -> p B H s t", B=2, H=2, s=1)
            nc.gpsimd.tensor_add(ya[:], u6[:, :, :, 0:1, :], u6[:, :, :, 1:2, :])
            nc.gpsimd.tensor_sub(ys[:], u6[:, :, :, 0:1, :], u6[:, :, :, 1:2, :])

            # ---- store (lagged) ----
            pending.append((y_st, r0))
            if len(pending) > LAG:
                y_prev, rp = pending.pop(0)
                nc.sync.dma_start(y_d.ap()[rp:rp + BLK, :], y_prev[:])

        for y_prev, rp in pending:
            nc.sync.dma_start(y_d.ap()[rp:rp + BLK, :], y_prev[:])


_NC = None


def _get_nc():
    global _NC
    if _NC is None:
        _NC = _build_nc()
    return _NC


def kernel(x, golay, alpha):
    global LAST_RESULT
    x_np = np.ascontiguousarray(np.asarray(x, dtype=np.float32).reshape(ROWS_TOTAL, DIM))
    golay_np = np.asarray(golay, dtype=np.float32).reshape(DIM)
    alpha_np = np.float32(np.asarray(alpha, dtype=np.float32))

    s = np.float32(1.0) / (alpha_np + np.float32(EPS))
    svec = np.full((128, 1), s, dtype=np.float32)
    h128 = _hadamard(128)
    # hm[b, a*128 + b'] = golay[a*128 + b] * H128[b, b']
    gmat = golay_np.reshape(8, 128)          # [a, b]
    # hm[b, a, b'] = gmat[a, b] * h128[b, b']
    hm = np.empty((128, 8, 128), dtype=np.float32)
    for a in range(8):
        hm[:, a, :] = gmat[a][:, None] * h128
    hm_bf16 = np.ascontiguousarray(hm.reshape(128, 8 * 128).astype(ml_dtypes.bfloat16))
    ident = np.ascontiguousarray(np.eye(128, dtype=np.float32))

    nc = _get_nc()
    in_maps = [
        {
            "x": x_np[c * ROWS_PER_CORE:(c + 1) * ROWS_PER_CORE],
            "svec": svec,
            "hmats": hm_bf16,
            "ident": ident,
        }
        for c in range(N_CORES)
    ]
    res = run_bass_kernel_spmd(nc, in_maps, core_ids=list(range(N_CORES)))
    LAST_RESULT = res
    y = np.concatenate([r["y"] for r in res.results], axis=0)
    return y.reshape(4, 8192, DIM)


# revision 13
# speedup vs baseline: 1.2091x; 1.1931x over previous
"""Trainium2 Bass kernel for nn_CompressModel: y = FHT_1024(x * golay) / (alpha + eps).

Factorization: H_1024 = H_8 (outer, feat bits 7-9) (x) H_128 (inner, feat bits 0-6).

Per-core dataflow (pure data-parallel over rows; 4096 rows/core, blocks of 128 rows):
  1. DMA x block [128 rows, 1024] -> SBUF [128p, 1024f]  (straight, 4KB descriptors)
  2. PE transposes each [128r, 128b] a-block -> PSUM zt [b, (a r)]  (2 banks of 4)
  3. ScalarE drains PSUM->SBUF bf16 with 1/(alpha+eps) per-partition scale
  4. PE matmul w_a[r,b'] = zb_a[b,r].T @ Ha[b,b'] in bf16 (1 cyc/row), where
     Ha = golay(a*128+b) * H128  -- golay multiply folded into 8 stationary mats
  5. H_8 = 3 butterfly levels over a-blocks in row space:
     L1 fused with PSUM drain on DVE (fp32->bf16), L2 on DVE (bf16 4x), L3 on
     GpSimd (bf16->fp32 out)
  6. DMA out.

All engines sit well below the ~93us/core DMA roofline (33.5 MB @ 360 GB/s), so
the 16 DMA engines stream continuously.
"""

import numpy as np
from contextlib import ExitStack

import ml_dtypes

import concourse.bass as bass
import concourse.tile as tile
from concourse import bacc, mybir
from concourse.bass_utils import run_bass_kernel_spmd

f32 = mybir.dt.float32
bf16 = mybir.dt.bfloat16

N_CORES = 8
DIM = 1024
EPS = 1e-5
ROWS_TOTAL = 4 * 8192          # 32768
ROWS_PER_CORE = ROWS_TOTAL // N_CORES   # 4096
BLK = 128                      # rows per block
N_BLK = ROWS_PER_CORE // BLK   # 32

LAST_RESULT = None  # test harness reads exec_time_ns from here


def _hadamard(n: int) -> np.ndarray:
    h = np.array([[1.0]], dtype=np.float32)
    while h.shape[0] < n:
        h = np.block([[h, h], [h, -h]])
    return np.ascontiguousarray(h.astype(np.float32))


def _build_nc():
    nc = bacc.Bacc("TRN2", target_bir_lowering=False, debug=False)
    x_d = nc.dram_tensor("x", [ROWS_PER_CORE, DIM], f32, kind="ExternalInput")
    s_d = nc.dram_tensor("svec", [128, 1], f32, kind="ExternalInput")
    h_d = nc.dram_tensor("hmats", [128, 8 * 128], bf16, kind="ExternalInput")
    i_d = nc.dram_tensor("ident", [128, 128], f32, kind="ExternalInput")
    y_d = nc.dram_tensor("y", [ROWS_PER_CORE, DIM], f32, kind="ExternalOutput")

    with TileKernel(nc) as tk:
        tk.emit(x_d, s_d, h_d, i_d, y_d)

    nc.compile()
    return nc


class TileKernel:
    def __init__(self, nc):
        self.nc = nc
        self.ctx = ExitStack()

    def __enter__(self):
        self.tc = self.ctx.enter_context(tile.TileContext(self.nc))
        return self

    def __exit__(self, *exc):
        return self.ctx.__exit__(*exc)

    def emit(self, x_d, s_d, h_d, i_d, y_d):
        nc, tc, ctx = self.nc, self.tc, self.ctx

        const_pool = ctx.enter_context(tc.tile_pool(name="const", bufs=1))
        x_pool = ctx.enter_context(tc.tile_pool(name="x", bufs=8))
        zb_pool = ctx.enter_context(tc.tile_pool(name="zb", bufs=4))
        v_pool = ctx.enter_context(tc.tile_pool(name="v", bufs=4))
        u_pool = ctx.enter_context(tc.tile_pool(name="u", bufs=4))
        w1s_pool = ctx.enter_context(tc.tile_pool(name="w1s", bufs=4))
        y_pool = ctx.enter_context(tc.tile_pool(name="y", bufs=8))
        zt_pool = ctx.enter_context(tc.tile_pool(name="zt", bufs=2, space="PSUM"))
        w_pool = ctx.enter_context(tc.tile_pool(name="w", bufs=2, space="PSUM"))

        ident = const_pool.tile([128, 128], f32)
        nc.sync.dma_start(ident[:], i_d.ap()[:, :])
        hm = const_pool.tile([128, 8 * 128], bf16)
        nc.sync.dma_start(hm[:], h_d.ap()[:, :])
        svec = const_pool.tile([128, 1], f32)
        nc.sync.dma_start(svec[:], s_d.ap()[:, :])

        # ---- software-pipelined emission ----
        # Each engine queue only sees ops whose producers ran >= 1 block
        # earlier, so no queue ever stalls mid-stream on a cross-engine dep.
        # Stage lags (block k's op runs at iteration k + lag):
        #   in-DMA 0 | transposes 1 | ztdrain 2 | matmuls 3 |
        #   w1s + L1 + L2 4 | L3 5 | out-DMA 6
        st = {}  # blk -> dict of live tiles

        def s_load(k):
            x_st = x_pool.tile([128, DIM], f32)
            nc.sync.dma_start(x_st[:], x_d.ap()[k * BLK:(k + 1) * BLK, :])
            st[k] = {"x": x_st}

        def s_transpose(k):
            zt = zt_pool.tile([128, DIM], f32)
            x_st = st[k].pop("x")
            for a in range(8):
                nc.tensor.transpose(
                    zt[:, a * 128:(a + 1) * 128],
                    x_st[:, a * 128:(a + 1) * 128],
                    ident[:],
                )
            st[k]["zt"] = zt

        def s_ztdrain(k):
            # PSUM -> SBUF bf16 with 1/(alpha+eps) scale
            zb = zb_pool.tile([128, DIM], bf16)
            nc.scalar.mul(zb[:], st[k].pop("zt")[:], svec[:, 0:1])
            st[k]["zb"] = zb

        def s_matmul(k):
            # inner H_128 (golay folded into Ha); even input blocks a ->
            # w0 slot a//2, odd -> w1, so the first butterfly level (over
            # a-bit0) pairs across the two PSUM tiles (DVE can read at most
            # one PSUM operand).
            zb = st[k].pop("zb")
            w0 = w_pool.tile([128, 512], f32)
            w1 = w_pool.tile([128, 512], f32)
            for a in range(8):
                w = w1 if a % 2 else w0
                j = a // 2
                nc.tensor.matmul(
                    w[:, j * 128:(j + 1) * 128],
                    lhsT=zb[:, a * 128:(a + 1) * 128],
                    rhs=hm[:, a * 128:(a + 1) * 128],
                    start=True, stop=True,
                )
            st[k]["w0"], st[k]["w1"] = w0, w1

        def s_butterfly12(k):
            w0, w1 = st[k].pop("w0"), st[k].pop("w1")
            # drain w1 -> SBUF bf16 on ACT (one-PSUM-operand rule)
            w1s = w1s_pool.tile([128, 512], bf16)
            nc.scalar.copy(w1s[:], w1[:])
            # level over a-bit0: v[H=sign][q] = w0[q] +/- w1s[q]; dense out
            v = v_pool.tile([128, DIM], bf16)
            nc.vector.tensor_add(v[:, 0:512], w0[:], w1s[:])
            nc.vector.tensor_sub(v[:, 512:1024], w0[:], w1s[:])
            # level over a-bit1: q=(A2,h) pairs along h; u[H][A2][B1=sign]
            u = u_pool.tile([128, DIM], bf16)
            v5 = v[:].rearrange("p (H A h t) -> p H A h t", H=2, A=2, h=2)
            u5 = u[:].rearrange("p (H A h t) -> p H A h t", H=2, A=2, h=2)
            nc.vector.tensor_add(u5[:, :, :, 0:1, :], v5[:, :, :, 0:1, :], v5[:, :, :, 1:2, :])
            nc.vector.tensor_sub(u5[:, :, :, 1:2, :], v5[:, :, :, 0:1, :], v5[:, :, :, 1:2, :])
            st[k]["u"] = u

        def s_butterfly3(k):
            # level over a-bit2: pairs along A2 (stride 256); y block index
            # a' = 4*s + 2*B1 + H, so each sign's output is a dense half.
            u = st[k].pop("u")
            y_st = y_pool.tile([128, DIM], f32)
            u6 = u[:].rearrange("p (H A B t) -> p B H A t", H=2, A=2, B=2)
            ya = y_st[:, 0:512].rearrange("p (B H s t) -> p B H s t", B=2, H=2, s=1)
            ys = y_st[:, 512:1024].rearrange("p (B H s t) -> p B H s t", B=2, H=2, s=1)
            nc.gpsimd.tensor_add(ya[:], u6[:, :, :, 0:1, :], u6[:, :, :, 1:2, :])
            nc.gpsimd.tensor_sub(ys[:], u6[:, :, :, 0:1, :], u6[:, :, :, 1:2, :])
            st[k]["y"] = y_st

        def s_store(k):
            y_st = st[k].pop("y")
            nc.sync.dma_start(y_d.ap()[k * BLK:(k + 1) * BLK, :], y_st[:])
            del st[k]

        stages = [s_load, s_transpose, s_ztdrain, s_matmul,
                  s_butterfly12, s_butterfly3, s_store]
        for ii in range(N_BLK + len(stages) - 1):
            for lag, fn in enumerate(stages):
                k = ii - lag
                if 0 <= k < N_BLK:
                    fn(k)


_NC = None


def _get_nc():
    global _NC
    if _NC is None:
        _NC = _build_nc()
    return _NC


def kernel(x, golay, alpha):
    global LAST_RESULT
    x_np = np.ascontiguousarray(np.asarray(x, dtype=np.float32).reshape(ROWS_TOTAL, DIM))
    golay_np = np.asarray(golay, dtype=np.float32).reshape(DIM)
    alpha_np = np.float32(np.asarray(alpha, dtype=np.float32))

    s = np.float32(1.0) / (alpha_np + np.float32(EPS))
    svec = np.full((128, 1), s, dtype=np.float32)
    h128 = _hadamard(128)
    # hm[b, a*128 + b'] = golay[a*128 + b] * H128[b, b']
    gmat = golay_np.reshape(8, 128)          # [a, b]
    # hm[b, a, b'] = gmat[a, b] * h128[b, b']
    hm = np.empty((128, 8, 128), dtype=np.float32)
    for a in range(8):
        hm[:, a, :] = gmat[a][:, None] * h128
    hm_bf16 = np.ascontiguousarray(hm.reshape(128, 8 * 128).astype(ml_dtypes.bfloat16))
    ident = np.ascontiguousarray(np.eye(128, dtype=np.float32))

    nc = _get_nc()
    in_maps = [
        {
            "x": x_np[c * ROWS_PER_CORE:(c + 1) * ROWS_PER_CORE],
            "svec": svec,
            "hmats": hm_bf16,
            "ident": ident,
        }
        for c in range(N_CORES)
    ]
    res = run_bass_kernel_spmd(nc, in_maps, core_ids=list(range(N_CORES)))
    LAST_RESULT = res
    y = np.concatenate([r["y"] for r in res.results], axis=0)
    return y.reshape(4, 8192, DIM)


# revision 15
# speedup vs baseline: 1.2169x; 1.0064x over previous
"""Trainium2 Bass kernel for nn_CompressModel: y = FHT_1024(x * golay) / (alpha + eps).

Factorization: H_1024 = H_8 (outer, feat bits 7-9) (x) H_128 (inner, feat bits 0-6).

Per-core dataflow (pure data-parallel over rows; 4096 rows/core, blocks of 128 rows):
  1. DMA x block [128 rows, 1024] -> SBUF [128p, 1024f]  (straight, 4KB descriptors)
  2. PE transposes each [128r, 128b] a-block -> PSUM zt [b, (a r)]  (2 banks of 4)
  3. ScalarE drains PSUM->SBUF bf16 with 1/(alpha+eps) per-partition scale
  4. PE matmul w_a[r,b'] = zb_a[b,r].T @ Ha[b,b'] in bf16 (1 cyc/row), where
     Ha = golay(a*128+b) * H128  -- golay multiply folded into 8 stationary mats
  5. H_8 = 3 butterfly levels over a-blocks in row space:
     L1 fused with PSUM drain on DVE (fp32->bf16), L2 on DVE (bf16 4x), L3 on
     GpSimd (bf16->fp32 out)
  6. DMA out.

All engines sit well below the ~93us/core DMA roofline (33.5 MB @ 360 GB/s), so
the 16 DMA engines stream continuously.
"""

import numpy as np
from contextlib import ExitStack

import ml_dtypes

import concourse.bass as bass
import concourse.tile as tile
from concourse import bacc, mybir
from concourse.bass_utils import run_bass_kernel_spmd

f32 = mybir.dt.float32
bf16 = mybir.dt.bfloat16

N_CORES = 8
DIM = 1024
EPS = 1e-5
ROWS_TOTAL = 4 * 8192          # 32768
ROWS_PER_CORE = ROWS_TOTAL // N_CORES   # 4096
BLK = 128                      # rows per block
N_BLK = ROWS_PER_CORE // BLK   # 32

LAST_RESULT = None  # test harness reads exec_time_ns from here


def _hadamard(n: int) -> np.ndarray:
    h = np.array([[1.0]], dtype=np.float32)
    while h.shape[0] < n:
        h = np.block([[h, h], [h, -h]])
    return np.ascontiguousarray(h.astype(np.float32))


def _build_nc():
    nc = bacc.Bacc("TRN2", target_bir_lowering=False, debug=False)
    x_d = nc.dram_tensor("x", [ROWS_PER_CORE, DIM], f32, kind="ExternalInput")
    s_d = nc.dram_tensor("svec", [128, 1], f32, kind="ExternalInput")
    h_d = nc.dram_tensor("hmats", [128, 8 * 128], bf16, kind="ExternalInput")
    i_d = nc.dram_tensor("ident", [128, 128], f32, kind="ExternalInput")
    y_d = nc.dram_tensor("y", [ROWS_PER_CORE, DIM], f32, kind="ExternalOutput")

    with TileKernel(nc) as tk:
        tk.emit(x_d, s_d, h_d, i_d, y_d)

    nc.compile()
    return nc


class TileKernel:
    def __init__(self, nc):
        self.nc = nc
        self.ctx = ExitStack()

    def __enter__(self):
        self.tc = self.ctx.enter_context(tile.TileContext(self.nc))
        return self

    def __exit__(self, *exc):
        return self.ctx.__exit__(*exc)

    def emit(self, x_d, s_d, h_d, i_d, y_d):
        nc, tc, ctx = self.nc, self.tc, self.ctx

        const_pool = ctx.enter_context(tc.tile_pool(name="const", bufs=1))
        x_pool = ctx.enter_context(tc.tile_pool(name="x", bufs=8))
        zb_pool = ctx.enter_context(tc.tile_pool(name="zb", bufs=4))
        v_pool = ctx.enter_context(tc.tile_pool(name="v", bufs=4))
        u_pool = ctx.enter_context(tc.tile_pool(name="u", bufs=4))
        w1s_pool = ctx.enter_context(tc.tile_pool(name="w1s", bufs=4))
        y_pool = ctx.enter_context(tc.tile_pool(name="y", bufs=8))
        zt_pool = ctx.enter_context(tc.tile_pool(name="zt", bufs=2, space="PSUM"))
        w_pool = ctx.enter_context(tc.tile_pool(name="w", bufs=2, space="PSUM"))

        ident = const_pool.tile([128, 128], f32)
        nc.sync.dma_start(ident[:], i_d.ap()[:, :])
        hm = const_pool.tile([128, 8 * 128], bf16)
        nc.sync.dma_start(hm[:], h_d.ap()[:, :])
        svec = const_pool.tile([128, 1], f32)
        nc.sync.dma_start(svec[:], s_d.ap()[:, :])

        # ---- software-pipelined emission ----
        # Each engine queue only sees ops whose producers ran >= 1 block
        # earlier, so no queue ever stalls mid-stream on a cross-engine dep.
        # Stage lags (block k's op runs at iteration k + lag):
        #   in-DMA 0 | transposes 1 | ztdrain 2 | matmuls 3 |
        #   w1s + L1 + L2 4 | L3 5 | out-DMA 6
        st = {}  # blk -> dict of live tiles

        def s_load(k):
            x_st = x_pool.tile([128, DIM], f32)
            nc.sync.dma_start(x_st[:], x_d.ap()[k * BLK:(k + 1) * BLK, :])
            st[k] = {"x": x_st}

        def s_transpose(k):
            zt = zt_pool.tile([128, DIM], f32)
            x_st = st[k].pop("x")
            for a in range(8):
                nc.tensor.transpose(
                    zt[:, a * 128:(a + 1) * 128],
                    x_st[:, a * 128:(a + 1) * 128],
                    ident[:],
                )
            st[k]["zt"] = zt

        def s_ztdrain(k):
            # PSUM -> SBUF bf16 with 1/(alpha+eps) scale
            zb = zb_pool.tile([128, DIM], bf16)
            nc.scalar.mul(zb[:], st[k].pop("zt")[:], svec[:, 0:1])
            st[k]["zb"] = zb

        def s_matmul(k):
            # inner H_128 (golay folded into Ha); even input blocks a ->
            # w0 slot a//2, odd -> w1, so the first butterfly level (over
            # a-bit0) pairs across the two PSUM tiles (DVE can read at most
            # one PSUM operand).
            zb = st[k].pop("zb")
            w0 = w_pool.tile([128, 512], f32)
            w1 = w_pool.tile([128, 512], f32)
            for a in range(8):
                w = w1 if a % 2 else w0
                j = a // 2
                nc.tensor.matmul(
                    w[:, j * 128:(j + 1) * 128],
                    lhsT=zb[:, a * 128:(a + 1) * 128],
                    rhs=hm[:, a * 128:(a + 1) * 128],
                    start=True, stop=True,
                )
            st[k]["w0"], st[k]["w1"] = w0, w1

        def s_w1s(k):
            # drain w1 -> SBUF bf16 on ACT (one-PSUM-operand rule); emitted
            # ahead of the ztdrain in the ACT queue so L1 never waits on it
            w1s = w1s_pool.tile([128, 512], bf16)
            nc.scalar.copy(w1s[:], st[k]["w1"][:])
            st[k]["w1s"] = w1s

        def s_butterfly12(k):
            w0 = st[k].pop("w0")
            st[k].pop("w1")
            w1s = st[k].pop("w1s")
            # level over a-bit0: v[H=sign][q] = w0[q] +/- w1s[q]; dense out
            v = v_pool.tile([128, DIM], bf16)
            nc.vector.tensor_add(v[:, 0:512], w0[:], w1s[:])
            nc.vector.tensor_sub(v[:, 512:1024], w0[:], w1s[:])
            # level over a-bit1: q=(A2,h) pairs along h; u[H][A2][B1=sign]
            u = u_pool.tile([128, DIM], bf16)
            v5 = v[:].rearrange("p (H A h t) -> p H A h t", H=2, A=2, h=2)
            u5 = u[:].rearrange("p (H A h t) -> p H A h t", H=2, A=2, h=2)
            nc.vector.tensor_add(u5[:, :, :, 0:1, :], v5[:, :, :, 0:1, :], v5[:, :, :, 1:2, :])
            nc.vector.tensor_sub(u5[:, :, :, 1:2, :], v5[:, :, :, 0:1, :], v5[:, :, :, 1:2, :])
            st[k]["u"] = u

        def s_butterfly3(k):
            # level over a-bit2: pairs along A2 (stride 256); y block index
            # a' = 4*s + 2*B1 + H, so each sign's output is a dense half.
            u = st[k].pop("u")
            y_st = y_pool.tile([128, DIM], f32)
            u6 = u[:].rearrange("p (H A B t) -> p B H A t", H=2, A=2, B=2)
            ya = y_st[:, 0:512].rearrange("p (B H s t) -> p B H s t", B=2, H=2, s=1)
            ys = y_st[:, 512:1024].rearrange("p (B H s t) -> p B H s t", B=2, H=2, s=1)
            nc.gpsimd.tensor_add(ya[:], u6[:, :, :, 0:1, :], u6[:, :, :, 1:2, :])
            nc.gpsimd.tensor_sub(ys[:], u6[:, :, :, 0:1, :], u6[:, :, :, 1:2, :])
            st[k]["y"] = y_st

        def s_store(k):
            y_st = st[k].pop("y")
            nc.sync.dma_start(y_d.ap()[k * BLK:(k + 1) * BLK, :], y_st[:])
            del st[k]

        stages = [(s_load, 0), (s_w1s, 4), (s_transpose, 1), (s_ztdrain, 2),
                  (s_matmul, 3), (s_butterfly12, 4), (s_butterfly3, 5),
                  (s_store, 6)]
        max_lag = max(lag for _, lag in stages)
        for ii in range(N_BLK + max_lag):
            for fn, lag in stages:
                k = ii - lag
                if 0 <= k < N_BLK:
                    fn(k)


_NC = None


def _get_nc():
    global _NC
    if _NC is None:
        _NC = _build_nc()
    return _NC


def kernel(x, golay, alpha):
    global LAST_RESULT
    x_np = np.ascontiguousarray(np.asarray(x, dtype=np.float32).reshape(ROWS_TOTAL, DIM))
    golay_np = np.asarray(golay, dtype=np.float32).reshape(DIM)
    alpha_np = np.float32(np.asarray(alpha, dtype=np.float32))

    s = np.float32(1.0) / (alpha_np + np.float32(EPS))
    svec = np.full((128, 1), s, dtype=np.float32)
    h128 = _hadamard(128)
    # hm[b, a*128 + b'] = golay[a*128 + b] * H128[b, b']
    gmat = golay_np.reshape(8, 128)          # [a, b]
    # hm[b, a, b'] = gmat[a, b] * h128[b, b']
    hm = np.empty((128, 8, 128), dtype=np.float32)
    for a in range(8):
        hm[:, a, :] = gmat[a][:, None] * h128
    hm_bf16 = np.ascontiguousarray(hm.reshape(128, 8 * 128).astype(ml_dtypes.bfloat16))
    ident = np.ascontiguousarray(np.eye(128, dtype=np.float32))

    nc = _get_nc()
    in_maps = [
        {
            "x": x_np[c * ROWS_PER_CORE:(c + 1) * ROWS_PER_CORE],
            "svec": svec,
            "hmats": hm_bf16,
            "ident": ident,
        }
        for c in range(N_CORES)
    ]
    res = run_bass_kernel_spmd(nc, in_maps, core_ids=list(range(N_CORES)))
    LAST_RESULT = res
    y = np.concatenate([r["y"] for r in res.results], axis=0)
    return y.reshape(4, 8192, DIM)
